# revision 23
# baseline (speedup 1.0000x reference)
"""Distributed Trainium2 kernel for nn_AddAttention_154618823089 — v3.

Computation (see reference):
    q = rope(bf16(hidden @ Wq.T)); k = rope(bf16(hidden @ Wk.T))
    o[b,l] = sum_{j<=l} exp(q_l . k_j / sqrt(DIM))          (no softmax norm)
    out = relu(o @ fc1_w.T + fc1_b) @ fc2_w.T + fc2_b

Sharding: every core c handles the strided row set {r : r % 8 == c} of
BOTH batches (512 rows each); K is exchanged via ONE ncfw AllGather.

v3 changes vs the 273us v1 baseline:
  - fp8(e4m3) K exchange and fp8 DoubleRow score matmuls: the q.k values
    are small (|q|,|k| < ~6), so fp8 quantization costs ~5e-3 rel err but
    halves both the score PE time (157 TF/s) and the gather/reload bytes.
  - ONE AllGather for both batches (1 MB in -> 8 MB out, fp8) triggered
    right at kernel start: the ncfw doorbell rings immediately, the mesh
    entry (~65-85us ncfw startup) fully overlaps the q/k projections, and
    the second-collective entry stall of v1 (data phase only at ~169us)
    disappears.
  - kt reload drops from 16 MB to 8 MB and SBUF pressure halves.
"""

import sys
import types

import numpy as np
from ml_dtypes import bfloat16, float8_e4m3fn

import concourse.bacc as bacc
import concourse.bass as bass
import concourse.mybir as mybir
import concourse.tile as tile
from concourse.bass_utils import run_bass_kernel_spmd


def _install_ntff_hook():
    """The container's antenv lacks axon_hooks; provide it so trace=True can
    capture NTFF profiles (exec_time_ns) through the axon PJRT library."""
    if "antenv.axon_hooks" in sys.modules:
        return
    try:
        sys.path.insert(0, "/root/.axon_site/trn_agent_boot")
        import trn_boot

        mod = types.ModuleType("antenv.axon_hooks")
        _h = {"hook": None}
        mod.set_axon_ntff_profile_hook = lambda h: _h.__setitem__("hook", h)
        mod.get_axon_ntff_profile_hook = lambda: _h["hook"]
        sys.modules["antenv.axon_hooks"] = mod
        import antenv

        antenv.axon_hooks = mod
        mod.set_axon_ntff_profile_hook(
            trn_boot._ntff_profile_via_ctypes("/opt/axon/libaxon_pjrt.so"))
    except Exception:
        pass


_install_ntff_hook()

B, L, DIM, INNER = 2, 4096, 1024, 16
ROPE_BASE = 32.0
NCORES = 8
RB = L // NCORES       # rows per core per batch (512)
RLOC = 2 * RB          # local q/k rows per core (both batches, 1024)
NSUB = RB // 128       # q subtiles per core per batch (4)
NDT = DIM // 128       # d tiles (8)
ND2 = NDT // 2         # fp8 DoubleRow d-tile pairs (4)
SCALE = 1.0 / float(np.sqrt(DIM))
MASK_NEG = -1.0e6
CHUNK = 3              # psum banks per score chunk
F32 = mybir.dt.float32
BF16 = mybir.dt.bfloat16
FP8 = mybir.dt.float8e4

_NC_CACHE = {}


def _build_nc():
    nc = bacc.Bacc("TRN2", target_bir_lowering=False, debug=False,
                   num_devices=NCORES, num_swdge_queues=4)

    hT8 = nc.dram_tensor("hT8", [128, ND2, 2, RLOC], FP8,
                         kind="ExternalInput")
    wq8_d = nc.dram_tensor("wq8", [128, ND2, 2, DIM], FP8,
                           kind="ExternalInput")
    wk8_d = nc.dram_tensor("wk8", [128, ND2, 2, DIM], FP8,
                           kind="ExternalInput")
    cosh = nc.dram_tensor("cosh", [DIM // 2, RLOC], BF16, kind="ExternalInput")
    sinh = nc.dram_tensor("sinh", [DIM // 2, RLOC], BF16, kind="ExternalInput")
    maskr_d = nc.dram_tensor("maskr", [128, NCORES, 128], F32,
                             kind="ExternalInput")
    w1b_d = nc.dram_tensor("w1b", [128, 32], F32, kind="ExternalInput")
    b1b_d = nc.dram_tensor("b1b", [128, 32], F32, kind="ExternalInput")
    w2aug = nc.dram_tensor("w2aug", [INNER + 1, DIM], BF16, kind="ExternalInput")
    onesrow = nc.dram_tensor("onesrow", [1, RB], BF16, kind="ExternalInput")
    out_d = nc.dram_tensor("out", [RLOC, DIM], F32, kind="ExternalOutput")

    # fp8 K bounce + gathered buffer: layout [p, b, dt2, two, j]
    kb8_d = nc.dram_tensor("kb8", [128, B, ND2, 2, RB], FP8)
    G = nc.dram_tensor("G", [NCORES * 128, B, ND2, 2, RB], FP8,
                       addr_space="Shared")

    groups = [list(range(NCORES))]

    with tile.TileContext(nc) as tc:
        with (
            tc.tile_pool(name="big", bufs=1) as big,
            tc.tile_pool(name="tmp", bufs=2) as tmp,
            tc.tile_pool(name="rsp", bufs=2) as rsp,
            tc.tile_pool(name="osb", bufs=2) as osbp,
            tc.tile_pool(name="ps", bufs=6, space="PSUM") as pps,
            tc.tile_pool(name="po", bufs=2, space="PSUM") as ppo,
        ):
            # ---- inputs -> SBUF, spread over queues ----
            h8_sb = big.tile([128, ND2, 2, RLOC], FP8, tag="h8")
            nc.sync.dma_start(h8_sb[:], hT8[:])
            wk8 = big.tile([128, ND2, 2, DIM], FP8, tag="wk8")
            nc.scalar.dma_start(wk8[:], wk8_d[:])
            wq8 = big.tile([128, ND2, 2, DIM], FP8, tag="wq8")
            nc.sync.dma_start(wq8[:], wq8_d[:])
            cos_t, sin_t = [], []
            for ci in range(NDT // 2):
                tc_ = big.tile([128, RLOC], BF16, tag=f"cos{ci}",
                               name=f"cos{ci}")
                nc.scalar.dma_start(tc_[:], cosh[128 * ci:128 * (ci + 1), :])
                cos_t.append(tc_)
                ts_ = big.tile([128, RLOC], BF16, tag=f"sin{ci}",
                               name=f"sin{ci}")
                nc.scalar.dma_start(ts_[:], sinh[128 * ci:128 * (ci + 1), :])
                sin_t.append(ts_)
            maskr_sb = big.tile([128, NCORES, 128], F32, tag="maskr")
            nc.scalar.dma_start(maskr_sb[:], maskr_d[:])
            w1b_sb = big.tile([128, 32], F32, tag="w1b")
            nc.scalar.dma_start(w1b_sb[:], w1b_d[:])
            b1b_sb = big.tile([128, 32], F32, tag="b1b")
            nc.scalar.dma_start(b1b_sb[:], b1b_d[:])
            w2_sb = big.tile([INNER + 1, DIM], BF16, tag="w2")
            nc.scalar.dma_start(w2_sb[:], w2aug[:])
            z_aug = big.tile([INNER + 1, RB], BF16, tag="zaug")
            nc.scalar.dma_start(z_aug[INNER:INNER + 1, :], onesrow[:])

            ks8 = big.tile([128, B, ND2, 2, RB], FP8, tag="ks8")
            q8 = big.tile([128, B, ND2, 2, RB], FP8, tag="q8")
            # gathered K: [p, b, rank, dt2, two, j] (rank-major: 4KB-contig
            # per-partition loads; scores slice [two, j-prefix] per rank)
            # rank INSIDE the free dims so one matmul can span a contiguous
            # rank group (s=0 packs 4 ranks, s=1 packs 2 -> far fewer
            # LDWEIGHTS+MATMUL issues on the PE sequencer)
            kt = big.tile([128, B, ND2, 2, NCORES, RB], FP8, tag="kt")

            def project_half(w8t, rt, f8):
                """fp8 DoubleRow projection of batch rt's columns; RoPE
                reads the f32 PSUM tiles directly and writes fp8 into
                f8[:, rt] (DoubleRow score layout).  No bf16 staging."""
                cols = slice(512 * rt, 512 * (rt + 1))
                live = {}

                def rope_pair(dt):
                    dh = dt + NDT // 2
                    cm = cos_t[dt][:, cols]
                    sm = sin_t[dt][:, cols]
                    lo = live.pop(dt)
                    hi = live.pop(dh)
                    f8lo = f8[:, rt, dt // 2, dt % 2, :]
                    f8hi = f8[:, rt, dh // 2, dh % 2, :]
                    ta = tmp.tile([128, 512], BF16, tag="ta", name="ta")
                    tb = tmp.tile([128, 512], BF16, tag="tb", name="tb")
                    td = tmp.tile([128, 512], BF16, tag="td", name="td")
                    nc.vector.tensor_mul(ta[:], lo[:], cm)
                    nc.vector.tensor_mul(tb[:], lo[:], sm)
                    nc.vector.tensor_mul(td[:], hi[:], sm)
                    nc.vector.tensor_sub(f8lo, ta[:], td[:])
                    nc.vector.tensor_mul(ta[:], hi[:], cm)
                    nc.vector.tensor_add(f8hi, ta[:], tb[:])

                order = [x for pair in zip(range(NDT // 2),
                                           range(NDT // 2, NDT))
                         for x in pair]            # 0,4,1,5,2,6,3,7
                for do in order:
                    ps = pps.tile([128, 512], F32, tag="ps",
                                  name=f"psp{rt}{do}")
                    for d2 in range(ND2):
                        nc.tensor.matmul(
                            ps[:], w8t[:, d2, :, 128 * do:128 * (do + 1)],
                            h8_sb[:, d2, :, cols],
                            start=(d2 == 0), stop=(d2 == ND2 - 1),
                            perf_mode=mybir.MatmulPerfMode.DoubleRow,
                        )
                    live[do] = ps
                    if do >= NDT // 2:
                        rope_pair(do - NDT // 2)

            # ---- k: project+rope+fp8 both batches, bounce, ONE AllGather.
            # Bounces go on the scalar HWDGE ring: the gpsimd queue reaches
            # the collective trigger immediately (~10us), so the ~47us ncfw
            # startup runs concurrently with the projections instead of
            # starting only once the bounce data is ready.
            project_half(wk8, 0, ks8)
            nc.scalar.dma_start(kb8_d[:, 0], ks8[:, 0])
            project_half(wk8, 1, ks8)
            nc.scalar.dma_start(kb8_d[:, 1], ks8[:, 1])
            nc.gpsimd.collective_compute(
                "AllGather", mybir.AluOpType.bypass, replica_groups=groups,
                ins=[kb8_d.ap().opt()], outs=[G.ap().opt()])

            # ---- q: project + rope + fp8 (overlaps the collective) ----
            project_half(wq8, 0, q8)
            project_half(wq8, 1, q8)

            # ---- gathered-K load: per (b, rank), b0 first.  Emitted after
            # the q work so the per-engine sequencers don't block on the
            # gather-complete semaphore before issuing the q casts.
            g_r = G.rearrange("(r p) b d t j -> r p b d t j", r=NCORES, p=128)
            _kteng = [nc.sync, nc.scalar, nc.gpsimd]
            for b in range(B):
                for r in range(NCORES):
                    eng = _kteng[(b * NCORES + r) % 3]
                    eng.dma_start(kt[:, b, :, :, r, :], g_r[r, :, b, :, :, :])

            o_sb = big.tile([128, B * NSUB], F32, tag="o")

            def scores(b):
                # per (s): all 8 ranks, k-col prefix 0..128(s+1) of each
                # rank's RB columns; diagonal 128-block gets the per-rank
                # additive mask; exp row-sums accumulate per group in rs_t.
                # Small-s rank blocks are packed into one PSUM tile so one
                # exp covers several ranks (fewer scalar ops).
                rs_t = [rsp.tile([128, NCORES], F32, tag=f"rs{s}",
                                 name=f"rs{b}{s}") for s in range(NSUB)]
                ngrp = [2, 4, 8, 8]
                chunk_of = [2, 4, 3, 3]   # tiles in flight; s<=1 covers all
                for s in range(NSUB):
                    w = 128 * (s + 1)
                    pack = NCORES // ngrp[s]
                    grps = [list(range(NCORES))[g * pack:(g + 1) * pack]
                            for g in range(ngrp[s])]
                    for c0 in range(0, len(grps), chunk_of[s]):
                        chunk = list(range(len(grps)))[c0:c0 + chunk_of[s]]
                        psl = [pps.tile([128, 512], F32, tag="ps",
                                        name=f"ps{b}{s}{c0}_{i}")
                               for i in range(len(chunk))]
                        for d2 in range(ND2):
                            lhsT = q8[:, b, d2, :, 128 * s:128 * (s + 1)]
                            for g, ps in zip(chunk, psl):
                                r0 = grps[g][0]
                                nc.tensor.matmul(
                                    ps[:, 0:pack * w], lhsT,
                                    kt[:, b, d2, :, r0:r0 + pack, 0:w],
                                    start=(d2 == 0),
                                    stop=(d2 == ND2 - 1),
                                    perf_mode=(
                                        mybir.MatmulPerfMode.DoubleRow),
                                )
                        for g, ps in zip(chunk, psl):
                            if s == 0:
                                # diag == whole block; 4 rank masks contig
                                nc.vector.tensor_add(
                                    ps[:, 0:512], ps[:, 0:512],
                                    maskr_sb[:, grps[g][0]:grps[g][0] + 4, :])
                            else:
                                for i, r in enumerate(grps[g]):
                                    nc.vector.tensor_add(
                                        ps[:, i * w + 128 * s:i * w + w],
                                        ps[:, i * w + 128 * s:i * w + w],
                                        maskr_sb[:, r, :])
                            nc.scalar.activation(
                                ps[:, 0:len(grps[g]) * w],
                                ps[:, 0:len(grps[g]) * w],
                                mybir.ActivationFunctionType.Exp,
                                scale=SCALE,
                                accum_out=rs_t[s][:, g:g + 1],
                            )
                for s in range(NSUB):
                    nc.vector.reduce_sum(
                        o_sb[:, NSUB * b + s:NSUB * b + s + 1],
                        rs_t[s][:, 0:ngrp[s]], axis=mybir.AxisListType.X)
                    mlp_sub(b, s)

            def mlp_sub(b, s):
                # o_sb[p, b*NSUB+s] is local row b*RB + 128s + p.
                # z[row, n] = relu(o[row]*w1[n] + b1[n]) with o as a
                # per-partition scalar, DVE-transposed into z_aug[n, row],
                # then out rows = z_aug.T @ w2aug for this subtile.
                col = NSUB * b + s
                zrow = tmp.tile([128, 32], F32, tag="zr", name=f"zr{b}{s}")
                nc.vector.tensor_scalar_mul(zrow[:], w1b_sb[:],
                                            o_sb[:, col:col + 1])
                nc.vector.tensor_add(zrow[:], zrow[:], b1b_sb[:])
                zrb = tmp.tile([128, 32], BF16, tag="zrb",
                               name=f"zrb{b}{s}")
                nc.vector.tensor_scalar_max(zrb[:], zrow[:], 0.0)
                zts = tmp.tile([32, 128], BF16, tag="zts", name=f"zts{b}{s}")
                for g in range(4):
                    nc.vector.transpose(zts[0:32, 32 * g:32 * (g + 1)],
                                        zrb[32 * g:32 * (g + 1), :])
                nc.vector.tensor_copy(z_aug[0:INNER, 128 * s:128 * (s + 1)],
                                      zts[0:INNER, :])
                ob = osbp.tile([128, DIM], F32, tag="ob", name=f"ob{b}{s}")
                for hh in range(2):
                    po = ppo.tile([128, 512], F32, tag="po",
                                  name=f"po{b}{s}{hh}")
                    nc.tensor.matmul(po[:],
                                     z_aug[:, 128 * s:128 * (s + 1)],
                                     w2_sb[:, 512 * hh:512 * (hh + 1)],
                                     start=True, stop=True)
                    nc.vector.tensor_copy(ob[:, 512 * hh:512 * (hh + 1)],
                                          po[:])
                row0 = RB * b + 128 * s
                eng = nc.gpsimd if s % 2 else nc.sync
                eng.dma_start(out_d[row0:row0 + 128, :], ob[:])

            scores(0)
            scores(1)

    nc.compile()
    return nc


def get_nc():
    if "nc" not in _NC_CACHE:
        _NC_CACHE["nc"] = _build_nc()
    return _NC_CACHE["nc"]


def make_in_maps(hidden_states, Wq, Wk, fc1_w, fc1_b, fc2_w, fc2_b):
    hidden_states = np.asarray(hidden_states, dtype=np.float32)
    Wq = np.asarray(Wq, dtype=np.float32)
    Wk = np.asarray(Wk, dtype=np.float32)
    fc1_w = np.asarray(fc1_w, dtype=np.float32)
    fc1_b = np.asarray(fc1_b, dtype=np.float32)
    fc2_w = np.asarray(fc2_w, dtype=np.float32)
    fc2_b = np.asarray(fc2_b, dtype=np.float32)

    def dr_layout(mT):
        # [1024 d_in, N] -> DoubleRow tile [128, 4, 2, N]
        return np.ascontiguousarray(
            mT.reshape(ND2, 2, 128, -1).transpose(2, 0, 1, 3))

    wq8 = dr_layout(Wq.T.astype(float8_e4m3fn))
    wk8 = dr_layout(Wk.T.astype(float8_e4m3fn))
    w1b = np.zeros((128, 32), dtype=np.float32)
    w1b[:, 0:INNER] = fc1_w.reshape(1, INNER)
    b1b = np.zeros((128, 32), dtype=np.float32)
    b1b[:, 0:INNER] = fc1_b.reshape(1, INNER)
    w2aug = np.concatenate([fc2_w.T, fc2_b[None, :]], axis=0).astype(bfloat16)

    inv_freq = ROPE_BASE ** (-np.arange(0, DIM, 2, dtype=np.float32) / DIM)

    in_maps = []
    for c in range(NCORES):
        rows = np.arange(RB) * NCORES + c            # global rows, per batch
        hT8 = dr_layout(np.concatenate(
            [hidden_states[b, rows, :].T for b in range(B)],
            axis=1).astype(float8_e4m3fn))           # [128, 4, 2, RLOC]
        ang = rows[:, None].astype(np.float32) * inv_freq[None, :]  # [RB,512]
        ch = np.cos(ang).T.astype(np.float32)        # [512, RB]
        sh = np.sin(ang).T.astype(np.float32)
        cosh = np.concatenate([ch, ch], axis=1).astype(bfloat16)
        sinh = np.concatenate([sh, sh], axis=1).astype(bfloat16)
        # maskr[p, r, t]: diagonal-block mask for rank r's k col t vs my q
        # row p (within-block): allow iff 8t + r <= 8p + c.
        p = np.arange(128)[:, None, None]
        t = np.arange(128)[None, None, :]
        r = np.arange(NCORES)[None, :, None]
        allow = (NCORES * t + r) <= (NCORES * p + c)
        maskr = np.where(allow, 0.0, MASK_NEG).astype(np.float32)
        in_maps.append({
            "hT8": hT8,
            "wq8": wq8, "wk8": wk8,
            "cosh": np.ascontiguousarray(cosh),
            "sinh": np.ascontiguousarray(sinh),
            "maskr": np.ascontiguousarray(maskr),
            "w1b": w1b, "b1b": b1b, "w2aug": w2aug,
            "onesrow": np.ones((1, RB), dtype=bfloat16),
        })
    return in_maps


def assemble_output(results):
    out = np.empty((B, L, DIM), dtype=np.float32)
    for c in range(NCORES):
        for b in range(B):
            out[b, c::NCORES, :] = results[c]["out"][RB * b:RB * (b + 1)]
    return out


def run(trace=False, **inputs):
    nc = get_nc()
    in_maps = make_in_maps(**inputs)
    res = run_bass_kernel_spmd(nc, in_maps, core_ids=list(range(NCORES)),
                               trace=trace)
    return assemble_output(res.results), res


def kernel(**inputs) -> np.ndarray:
    out, _ = run(trace=False, **inputs)
    return out


# revision 25
# speedup vs baseline: 1.1219x; 1.1219x over previous
"""Distributed Trainium2 kernel for nn_AddAttention_154618823089 — v3.

Computation (see reference):
    q = rope(bf16(hidden @ Wq.T)); k = rope(bf16(hidden @ Wk.T))
    o[b,l] = sum_{j<=l} exp(q_l . k_j / sqrt(DIM))          (no softmax norm)
    out = relu(o @ fc1_w.T + fc1_b) @ fc2_w.T + fc2_b

Sharding: every core c handles the strided row set {r : r % 8 == c} of
BOTH batches (512 rows each); K is exchanged via ONE ncfw AllGather.

v3 changes vs the 273us v1 baseline:
  - fp8(e4m3) K exchange and fp8 DoubleRow score matmuls: the q.k values
    are small (|q|,|k| < ~6), so fp8 quantization costs ~5e-3 rel err but
    halves both the score PE time (157 TF/s) and the gather/reload bytes.
  - ONE AllGather for both batches (1 MB in -> 8 MB out, fp8) triggered
    right at kernel start: the ncfw doorbell rings immediately, the mesh
    entry (~65-85us ncfw startup) fully overlaps the q/k projections, and
    the second-collective entry stall of v1 (data phase only at ~169us)
    disappears.
  - kt reload drops from 16 MB to 8 MB and SBUF pressure halves.
"""

import sys
import types

import numpy as np
from ml_dtypes import bfloat16, float8_e4m3fn

import concourse.bacc as bacc
import concourse.bass as bass
import concourse.mybir as mybir
import concourse.tile as tile
from concourse.bass_utils import run_bass_kernel_spmd


def _install_ntff_hook():
    """The container's antenv lacks axon_hooks; provide it so trace=True can
    capture NTFF profiles (exec_time_ns) through the axon PJRT library."""
    if "antenv.axon_hooks" in sys.modules:
        return
    try:
        sys.path.insert(0, "/root/.axon_site/trn_agent_boot")
        import trn_boot

        mod = types.ModuleType("antenv.axon_hooks")
        _h = {"hook": None}
        mod.set_axon_ntff_profile_hook = lambda h: _h.__setitem__("hook", h)
        mod.get_axon_ntff_profile_hook = lambda: _h["hook"]
        sys.modules["antenv.axon_hooks"] = mod
        import antenv

        antenv.axon_hooks = mod
        mod.set_axon_ntff_profile_hook(
            trn_boot._ntff_profile_via_ctypes("/opt/axon/libaxon_pjrt.so"))
    except Exception:
        pass


_install_ntff_hook()

B, L, DIM, INNER = 2, 4096, 1024, 16
ROPE_BASE = 32.0
NCORES = 8
RB = L // NCORES       # rows per core per batch (512)
RLOC = 2 * RB          # local q/k rows per core (both batches, 1024)
NSUB = RB // 128       # q subtiles per core per batch (4)
NDT = DIM // 128       # d tiles (8)
ND2 = NDT // 2         # fp8 DoubleRow d-tile pairs (4)
SCALE = 1.0 / float(np.sqrt(DIM))
MASK_NEG = -1.0e6
CHUNK = 3              # psum banks per score chunk
F32 = mybir.dt.float32
BF16 = mybir.dt.bfloat16
FP8 = mybir.dt.float8e4

_NC_CACHE = {}


def _build_nc():
    nc = bacc.Bacc("TRN2", target_bir_lowering=False, debug=False,
                   num_devices=NCORES, num_swdge_queues=4)

    hT8 = nc.dram_tensor("hT8", [128, ND2, 2, RLOC], FP8,
                         kind="ExternalInput")
    wq8_d = nc.dram_tensor("wq8", [128, ND2, 2, DIM], FP8,
                           kind="ExternalInput")
    wk8_d = nc.dram_tensor("wk8", [128, ND2, 2, DIM], FP8,
                           kind="ExternalInput")
    cosh = nc.dram_tensor("cosh", [DIM // 2, RLOC], BF16, kind="ExternalInput")
    sinh = nc.dram_tensor("sinh", [DIM // 2, RLOC], BF16, kind="ExternalInput")
    maskr_d = nc.dram_tensor("maskr", [128, NCORES, 128], F32,
                             kind="ExternalInput")
    w1b_d = nc.dram_tensor("w1b", [128, 32], F32, kind="ExternalInput")
    b1b_d = nc.dram_tensor("b1b", [128, 32], F32, kind="ExternalInput")
    w2aug = nc.dram_tensor("w2aug", [INNER + 1, DIM], BF16, kind="ExternalInput")
    onesrow = nc.dram_tensor("onesrow", [1, RB], BF16, kind="ExternalInput")
    out_d = nc.dram_tensor("out", [RLOC, DIM], F32, kind="ExternalOutput")

    # fp8 K bounce + gathered buffer: layout [p, b, dt2, two, j]
    kb8_d = nc.dram_tensor("kb8", [128, B, ND2, 2, RB], FP8)
    G = nc.dram_tensor("G", [NCORES * 128, B, ND2, 2, RB], FP8,
                       addr_space="Shared")

    groups = [list(range(NCORES))]

    with tile.TileContext(nc) as tc:
        with (
            tc.tile_pool(name="big", bufs=1) as big,
            tc.tile_pool(name="tmp", bufs=2) as tmp,
            tc.tile_pool(name="rsp", bufs=2) as rsp,
            tc.tile_pool(name="osb", bufs=2) as osbp,
            tc.tile_pool(name="ps", bufs=6, space="PSUM") as pps,
            tc.tile_pool(name="po", bufs=2, space="PSUM") as ppo,
        ):
            # ---- inputs -> SBUF, spread over queues ----
            h8_sb = big.tile([128, ND2, 2, RLOC], FP8, tag="h8")
            nc.sync.dma_start(h8_sb[:], hT8[:])
            wk8 = big.tile([128, ND2, 2, DIM], FP8, tag="wk8")
            nc.scalar.dma_start(wk8[:], wk8_d[:])
            wq8 = big.tile([128, ND2, 2, DIM], FP8, tag="wq8")
            nc.sync.dma_start(wq8[:], wq8_d[:])
            cos_t, sin_t = [], []
            for ci in range(NDT // 2):
                tc_ = big.tile([128, RLOC], BF16, tag=f"cos{ci}",
                               name=f"cos{ci}")
                nc.scalar.dma_start(tc_[:], cosh[128 * ci:128 * (ci + 1), :])
                cos_t.append(tc_)
                ts_ = big.tile([128, RLOC], BF16, tag=f"sin{ci}",
                               name=f"sin{ci}")
                nc.scalar.dma_start(ts_[:], sinh[128 * ci:128 * (ci + 1), :])
                sin_t.append(ts_)
            maskr_sb = big.tile([128, NCORES, 128], F32, tag="maskr")
            nc.scalar.dma_start(maskr_sb[:], maskr_d[:])
            w1b_sb = big.tile([128, 32], F32, tag="w1b")
            nc.scalar.dma_start(w1b_sb[:], w1b_d[:])
            b1b_sb = big.tile([128, 32], F32, tag="b1b")
            nc.scalar.dma_start(b1b_sb[:], b1b_d[:])
            w2_sb = big.tile([INNER + 1, DIM], BF16, tag="w2")
            nc.scalar.dma_start(w2_sb[:], w2aug[:])
            z_aug = big.tile([INNER + 1, RB], BF16, tag="zaug")
            nc.scalar.dma_start(z_aug[INNER:INNER + 1, :], onesrow[:])

            ks8 = big.tile([128, B, ND2, 2, RB], FP8, tag="ks8")
            q8 = big.tile([128, B, ND2, 2, RB], FP8, tag="q8")
            # gathered K: [p, b, rank, dt2, two, j] (rank-major: 4KB-contig
            # per-partition loads; scores slice [two, j-prefix] per rank)
            kt = big.tile([128, B, NCORES, ND2, 2, RB], FP8, tag="kt")

            def project_half(w8t, rt, f8):
                """fp8 DoubleRow projection of batch rt's columns; RoPE
                reads the f32 PSUM tiles directly and writes fp8 into
                f8[:, rt] (DoubleRow score layout).  No bf16 staging."""
                cols = slice(512 * rt, 512 * (rt + 1))
                live = {}

                def rope_pair(dt):
                    dh = dt + NDT // 2
                    cm = cos_t[dt][:, cols]
                    sm = sin_t[dt][:, cols]
                    lo = live.pop(dt)
                    hi = live.pop(dh)
                    f8lo = f8[:, rt, dt // 2, dt % 2, :]
                    f8hi = f8[:, rt, dh // 2, dh % 2, :]
                    ta = tmp.tile([128, 512], BF16, tag="ta", name="ta")
                    tb = tmp.tile([128, 512], BF16, tag="tb", name="tb")
                    td = tmp.tile([128, 512], BF16, tag="td", name="td")
                    nc.vector.tensor_mul(ta[:], lo[:], cm)
                    nc.vector.tensor_mul(tb[:], lo[:], sm)
                    nc.vector.tensor_mul(td[:], hi[:], sm)
                    nc.vector.tensor_sub(f8lo, ta[:], td[:])
                    nc.vector.tensor_mul(ta[:], hi[:], cm)
                    nc.vector.tensor_add(f8hi, ta[:], tb[:])

                order = [x for pair in zip(range(NDT // 2),
                                           range(NDT // 2, NDT))
                         for x in pair]            # 0,4,1,5,2,6,3,7
                for do in order:
                    ps = pps.tile([128, 512], F32, tag="ps",
                                  name=f"psp{rt}{do}")
                    for d2 in range(ND2):
                        nc.tensor.matmul(
                            ps[:], w8t[:, d2, :, 128 * do:128 * (do + 1)],
                            h8_sb[:, d2, :, cols],
                            start=(d2 == 0), stop=(d2 == ND2 - 1),
                            perf_mode=mybir.MatmulPerfMode.DoubleRow,
                        )
                    live[do] = ps
                    if do >= NDT // 2:
                        rope_pair(do - NDT // 2)

            # ---- k: project+rope+fp8 both batches, bounce, ONE AllGather.
            # Bounces go on the scalar HWDGE ring: the gpsimd queue reaches
            # the collective trigger immediately (~10us), so the ~47us ncfw
            # startup runs concurrently with the projections instead of
            # starting only once the bounce data is ready.
            project_half(wk8, 0, ks8)
            nc.scalar.dma_start(kb8_d[:, 0], ks8[:, 0])
            project_half(wk8, 1, ks8)
            nc.scalar.dma_start(kb8_d[:, 1], ks8[:, 1])
            nc.gpsimd.collective_compute(
                "AllGather", mybir.AluOpType.bypass, replica_groups=groups,
                ins=[kb8_d.ap().opt()], outs=[G.ap().opt()])

            # ---- q: project + rope + fp8 (overlaps the collective) ----
            project_half(wq8, 0, q8)
            project_half(wq8, 1, q8)

            # ---- gathered-K load: per (b, rank), b0 first.  Emitted after
            # the q work so the per-engine sequencers don't block on the
            # gather-complete semaphore before issuing the q casts.
            g_r = G.rearrange("(r p) b d t j -> r p b d t j", r=NCORES, p=128)
            _kteng = [nc.sync, nc.scalar, nc.gpsimd]
            for b in range(B):
                for r in range(NCORES):
                    eng = _kteng[(b * NCORES + r) % 3]
                    eng.dma_start(kt[:, b, r, :, :, :], g_r[r, :, b, :, :, :])

            o_sb = big.tile([128, B * NSUB], F32, tag="o")

            def scores(b):
                # per (s): all 8 ranks, k-col prefix 0..128(s+1) of each
                # rank's RB columns; diagonal 128-block gets the per-rank
                # additive mask; exp row-sums accumulate per group in rs_t.
                # Small-s rank blocks are packed into one PSUM tile so one
                # exp covers several ranks (fewer scalar ops).
                rs_t = [rsp.tile([128, NCORES], F32, tag=f"rs{s}",
                                 name=f"rs{b}{s}") for s in range(NSUB)]
                ngrp = [2, 4, 8, 8]
                chunk_of = [2, 4, 4, 4]   # tiles in flight; s<=1 covers all
                for s in range(NSUB):
                    w = 128 * (s + 1)
                    pack = NCORES // ngrp[s]
                    grps = [list(range(NCORES))[g * pack:(g + 1) * pack]
                            for g in range(ngrp[s])]
                    for c0 in range(0, len(grps), chunk_of[s]):
                        chunk = list(range(len(grps)))[c0:c0 + chunk_of[s]]
                        psl = [pps.tile([128, 512], F32, tag="ps",
                                        name=f"ps{b}{s}{c0}_{i}")
                               for i in range(len(chunk))]
                        for d2 in range(ND2):
                            lhsT = q8[:, b, d2, :, 128 * s:128 * (s + 1)]
                            for g, ps in zip(chunk, psl):
                                for i, r in enumerate(grps[g]):
                                    nc.tensor.matmul(
                                        ps[:, i * w:i * w + w], lhsT,
                                        kt[:, b, r, d2, :, 0:w],
                                        start=(d2 == 0),
                                        stop=(d2 == ND2 - 1),
                                        perf_mode=(
                                            mybir.MatmulPerfMode.DoubleRow),
                                    )
                        for g, ps in zip(chunk, psl):
                            if s == 0:
                                # diag == whole block; 4 rank masks contig
                                nc.vector.tensor_add(
                                    ps[:, 0:512], ps[:, 0:512],
                                    maskr_sb[:, grps[g][0]:grps[g][0] + 4, :])
                            else:
                                for i, r in enumerate(grps[g]):
                                    nc.vector.tensor_add(
                                        ps[:, i * w + 128 * s:i * w + w],
                                        ps[:, i * w + 128 * s:i * w + w],
                                        maskr_sb[:, r, :])
                            nc.scalar.activation(
                                ps[:, 0:len(grps[g]) * w],
                                ps[:, 0:len(grps[g]) * w],
                                mybir.ActivationFunctionType.Exp,
                                scale=SCALE,
                                accum_out=rs_t[s][:, g:g + 1],
                            )
                for s in range(NSUB):
                    nc.vector.reduce_sum(
                        o_sb[:, NSUB * b + s:NSUB * b + s + 1],
                        rs_t[s][:, 0:ngrp[s]], axis=mybir.AxisListType.X)
                    mlp_sub(b, s)

            def mlp_sub(b, s):
                # o_sb[p, b*NSUB+s] is local row b*RB + 128s + p.
                # z[row, n] = relu(o[row]*w1[n] + b1[n]) with o as a
                # per-partition scalar, DVE-transposed into z_aug[n, row],
                # then out rows = z_aug.T @ w2aug for this subtile.
                col = NSUB * b + s
                zrow = tmp.tile([128, 32], F32, tag="zr", name=f"zr{b}{s}")
                nc.vector.tensor_scalar_mul(zrow[:], w1b_sb[:],
                                            o_sb[:, col:col + 1])
                nc.vector.tensor_add(zrow[:], zrow[:], b1b_sb[:])
                zrb = tmp.tile([128, 32], BF16, tag="zrb",
                               name=f"zrb{b}{s}")
                nc.vector.tensor_scalar_max(zrb[:], zrow[:], 0.0)
                zts = tmp.tile([32, 128], BF16, tag="zts", name=f"zts{b}{s}")
                for g in range(4):
                    nc.vector.transpose(zts[0:32, 32 * g:32 * (g + 1)],
                                        zrb[32 * g:32 * (g + 1), :])
                nc.vector.tensor_copy(z_aug[0:INNER, 128 * s:128 * (s + 1)],
                                      zts[0:INNER, :])
                ob = osbp.tile([128, DIM], F32, tag="ob", name=f"ob{b}{s}")
                for hh in range(2):
                    po = ppo.tile([128, 512], F32, tag="po",
                                  name=f"po{b}{s}{hh}")
                    nc.tensor.matmul(po[:],
                                     z_aug[:, 128 * s:128 * (s + 1)],
                                     w2_sb[:, 512 * hh:512 * (hh + 1)],
                                     start=True, stop=True)
                    nc.vector.tensor_copy(ob[:, 512 * hh:512 * (hh + 1)],
                                          po[:])
                row0 = RB * b + 128 * s
                eng = nc.gpsimd if s % 2 else nc.sync
                eng.dma_start(out_d[row0:row0 + 128, :], ob[:])

            scores(0)
            scores(1)

    nc.compile()
    return nc


def get_nc():
    if "nc" not in _NC_CACHE:
        _NC_CACHE["nc"] = _build_nc()
    return _NC_CACHE["nc"]


def make_in_maps(hidden_states, Wq, Wk, fc1_w, fc1_b, fc2_w, fc2_b):
    hidden_states = np.asarray(hidden_states, dtype=np.float32)
    Wq = np.asarray(Wq, dtype=np.float32)
    Wk = np.asarray(Wk, dtype=np.float32)
    fc1_w = np.asarray(fc1_w, dtype=np.float32)
    fc1_b = np.asarray(fc1_b, dtype=np.float32)
    fc2_w = np.asarray(fc2_w, dtype=np.float32)
    fc2_b = np.asarray(fc2_b, dtype=np.float32)

    def dr_layout(mT):
        # [1024 d_in, N] -> DoubleRow tile [128, 4, 2, N]
        return np.ascontiguousarray(
            mT.reshape(ND2, 2, 128, -1).transpose(2, 0, 1, 3))

    wq8 = dr_layout(Wq.T.astype(float8_e4m3fn))
    wk8 = dr_layout(Wk.T.astype(float8_e4m3fn))
    w1b = np.zeros((128, 32), dtype=np.float32)
    w1b[:, 0:INNER] = fc1_w.reshape(1, INNER)
    b1b = np.zeros((128, 32), dtype=np.float32)
    b1b[:, 0:INNER] = fc1_b.reshape(1, INNER)
    w2aug = np.concatenate([fc2_w.T, fc2_b[None, :]], axis=0).astype(bfloat16)

    inv_freq = ROPE_BASE ** (-np.arange(0, DIM, 2, dtype=np.float32) / DIM)

    in_maps = []
    for c in range(NCORES):
        rows = np.arange(RB) * NCORES + c            # global rows, per batch
        hT8 = dr_layout(np.concatenate(
            [hidden_states[b, rows, :].T for b in range(B)],
            axis=1).astype(float8_e4m3fn))           # [128, 4, 2, RLOC]
        ang = rows[:, None].astype(np.float32) * inv_freq[None, :]  # [RB,512]
        ch = np.cos(ang).T.astype(np.float32)        # [512, RB]
        sh = np.sin(ang).T.astype(np.float32)
        cosh = np.concatenate([ch, ch], axis=1).astype(bfloat16)
        sinh = np.concatenate([sh, sh], axis=1).astype(bfloat16)
        # maskr[p, r, t]: diagonal-block mask for rank r's k col t vs my q
        # row p (within-block): allow iff 8t + r <= 8p + c.
        p = np.arange(128)[:, None, None]
        t = np.arange(128)[None, None, :]
        r = np.arange(NCORES)[None, :, None]
        allow = (NCORES * t + r) <= (NCORES * p + c)
        maskr = np.where(allow, 0.0, MASK_NEG).astype(np.float32)
        in_maps.append({
            "hT8": hT8,
            "wq8": wq8, "wk8": wk8,
            "cosh": np.ascontiguousarray(cosh),
            "sinh": np.ascontiguousarray(sinh),
            "maskr": np.ascontiguousarray(maskr),
            "w1b": w1b, "b1b": b1b, "w2aug": w2aug,
            "onesrow": np.ones((1, RB), dtype=bfloat16),
        })
    return in_maps


def assemble_output(results):
    out = np.empty((B, L, DIM), dtype=np.float32)
    for c in range(NCORES):
        for b in range(B):
            out[b, c::NCORES, :] = results[c]["out"][RB * b:RB * (b + 1)]
    return out


def run(trace=False, **inputs):
    nc = get_nc()
    in_maps = make_in_maps(**inputs)
    res = run_bass_kernel_spmd(nc, in_maps, core_ids=list(range(NCORES)),
                               trace=trace)
    return assemble_output(res.results), res


def kernel(**inputs) -> np.ndarray:
    out, _ = run(trace=False, **inputs)
    return out
